# revision 9
# baseline (speedup 1.0000x reference)
"""Multi-head attention with RoPE - Trainium2 Bass/Tile kernel.

Problem (hardcoded): B=2, S=2048, D=1024, H=16 heads, d_k=64, causal,
RoPE (theta=10000) on Q/K, fp32 reference.

Sharding: 8 cores = 2 batches x 4 head-groups. Each core: QKV projections
for its 4 heads, RoPE, causal attention, o_proj row-slice -> partial [S, D];
host gather sums 4 partials per batch.

v2 over the baseline:
  - Streamed pipeline: x arrives in 512-col s-chunks; projections+RoPE per
    chunk; attention per q-chunk; o_proj + output DMA per q-chunk (no
    serial phases, no output tail).
  - bf16 on the attention path (rotq/rotk, V, exp output, tril mask,
    attnT, wo): score/PV matmuls run 1 cyc/col at ANY width, so the
    causal-diagonal matmuls are width-trimmed instead of paying the
    fp32r <256-col 4x penalty.
  - PSUM: one shared ring tag (2x [128,1024] = 4 banks) for proj/score/
    o_proj tiles + 2x2 banks PV accumulators = exactly 8 banks.
  - o_proj psum->sbuf copies on the (idle) Pool engine; exp stays on ACT;
    reciprocal + RoPE + normalize on DVE.
  - Units are atomic per psum tile (all writers+readers emitted before the
    ring wraps) to avoid in-order engine-queue deadlocks; chunk qc+1
    projections are woven into the tail of chunk qc's attention.
"""

import numpy as np

import concourse.tile as tile
from concourse import bacc, mybir
from concourse.bass_utils import run_bass_kernel_spmd

F32 = mybir.dt.float32
F32R = mybir.dt.float32r
BF16 = mybir.dt.bfloat16
EXP = mybir.ActivationFunctionType.Exp
COPY = mybir.ActivationFunctionType.Copy

B, S, D, H, DK = 2, 2048, 1024, 16, 64
P = 128
NCORES = 8
HPC = 4  # heads per core
GD = HPC * DK  # 256 head dims per core
NDT = D // P  # 8 d-tiles
NST = S // P  # 16 s-tiles
CH = 512  # q/s chunk
NQC = S // CH  # 4 chunks
KPC = CH // P  # 4 k-tiles per chunk
NSQ = NST // NQC  # 4 s-tiles per chunk
THETA = 10000.0
SCALE = 1.0 / 8.0  # 1/sqrt(DK)
SHUF_MASK = [(i + 16) % 32 for i in range(32)]

_CACHE = {}


def _build_nc():
    nc = bacc.Bacc("TRN2", target_bir_lowering=False, debug=False)
    x = nc.dram_tensor("x", [D, S], BF16, kind="ExternalInput").ap()
    wq = nc.dram_tensor("wq", [D, GD], BF16, kind="ExternalInput").ap()
    wk = nc.dram_tensor("wk", [D, GD], BF16, kind="ExternalInput").ap()
    wv = nc.dram_tensor("wv", [D, GD], BF16, kind="ExternalInput").ap()
    wo = nc.dram_tensor("wo", [GD, D], BF16, kind="ExternalInput").ap()
    cosf = nc.dram_tensor("cosf", [P, S], BF16, kind="ExternalInput").ap()
    sinf = nc.dram_tensor("sinf", [P, S], BF16, kind="ExternalInput").ap()
    dmask = nc.dram_tensor("dmask", [P, P], BF16, kind="ExternalInput").ap()
    out = nc.dram_tensor("out", [S, D], BF16, kind="ExternalOutput").ap()

    with tile.TileContext(nc) as tc:
        with (
            tc.tile_pool(name="sb", bufs=1) as sb,
            tc.tile_pool(name="rg", bufs=1) as rg,
            tc.tile_pool(name="ps", bufs=1, space="PSUM") as ps,
        ):
            # ---- persistent SBUF tiles
            xt = [
                sb.tile([P, S], BF16, tag=f"xt{dt}", name=f"xt{dt}")
                for dt in range(NDT)
            ]
            wvT = sb.tile([P, NDT, GD], BF16, tag="wv", name="wvT")
            wkT = sb.tile([P, NDT, GD], BF16, tag="wk", name="wkT")
            wqT = sb.tile([P, NDT, GD], BF16, tag="wq", name="wqT")
            cosT = sb.tile([P, S], BF16, tag="cos", name="cosT")
            sinT = sb.tile([P, S], BF16, tag="sin", name="sinT")
            dmT = sb.tile([P, P], BF16, tag="dm", name="dmT")
            woT = sb.tile([P, 2, D], BF16, tag="wo", name="woT")
            rotq = [
                sb.tile([P, S], BF16, tag=f"rotq{i}", name=f"rotq{i}")
                for i in range(2)
            ]
            rotk = [
                sb.tile([P, S], BF16, tag=f"rotk{i}", name=f"rotk{i}")
                for i in range(2)
            ]
            vt = [
                sb.tile([P, NSQ, HPC, 2 * DK], BF16, tag=f"vt{qq}", name=f"vt{qq}")
                for qq in range(NQC)
            ]

            # ---- input DMA emission (SP queue processes in this order).
            # x chunk 0 right after wv so V-proj matmuls start ASAP,
            # staggered per d-tile; cos/sin before the x tail so RoPE(0)
            # isn't blocked; wo late (first needed by o_proj ~25us in).
            xr = x.rearrange("(dt p) s -> dt p s", p=P)

            def dma_x_chunk(c):
                for dt in range(NDT):
                    nc.sync.dma_start(
                        xt[dt][:, c * CH : (c + 1) * CH], xr[dt][:, c * CH : (c + 1) * CH]
                    )

            def dma_cs_half(h):
                sl = slice(h * S // 2, (h + 1) * S // 2)
                nc.sync.dma_start(cosT[:, sl], cosf[:, sl])
                nc.sync.dma_start(sinT[:, sl], sinf[:, sl])

            wkr = wk.rearrange("(dt p) i -> p dt i", p=P)
            nc.sync.dma_start(wkT[:, 0:1, :], wkr[:, 0:1, :])
            nc.sync.dma_start(wkT[:, 1:NDT, :], wkr[:, 1:NDT, :])
            nc.sync.dma_start(
                wqT[:], wq.rearrange("(dt p) i -> p dt i", p=P)
            )
            dma_x_chunk(0)
            dma_cs_half(0)
            nc.sync.dma_start(dmT[:], dmask)
            nc.sync.dma_start(
                wvT[:], wv.rearrange("(dt p) i -> p dt i", p=P)
            )
            dma_x_chunk(1)
            dma_x_chunk(2)
            dma_cs_half(1)
            nc.sync.dma_start(woT[:], wo.rearrange("(it p) j -> p it j", p=P))
            dma_x_chunk(3)

            # softmax-denominator ones columns (Pool; disjoint from the
            # value columns the ACT copies write)
            for qq in range(NQC):
                nc.gpsimd.memset(vt[qq][:, :, :, 0:DK], 1.0)

            attn_tiles = {}

            # PE warm-up: junk matmuls on dmT (tiny first DMA) into a
            # scratch pso-slot keep the PE clock ramped through the weight/x0
            # DMA lead-in, so the first real projections run at full rate.
            warm = ps.tile([P, 2 * CH], F32, tag="pso", bufs=2, name="warm")
            for i in range(45):
                nc.tensor.matmul(
                    warm[:, 0:GD],
                    wkT[:, 0, 0:P],
                    wkT[:, 0, :],
                    start=True,
                    stop=True,
                )

            # ---- unit builders (each call EMITS instructions; units are
            # atomic wrt their psum work tile)
            def unit_V(qc):
                def u():
                    qoff = qc * CH
                    psv = ps.tile(
                        [P, NSQ * GD], F32, tag="work", bufs=2, name=f"psv{qc}"
                    )
                    # st-major: psum allows only ONE pending accumulation
                    # group per bank zero-region
                    for st4 in range(NSQ):
                        for dt in range(NDT):
                            nc.tensor.matmul(
                                psv[:, st4 * GD : (st4 + 1) * GD],
                                xt[dt][:, qoff + st4 * P : qoff + (st4 + 1) * P],
                                wvT[:, dt, :],
                                start=(dt == 0),
                                stop=(dt == NDT - 1),
                            )
                    nc.scalar.activation(
                        vt[qc][:, :, :, DK : 2 * DK],
                        psv[:].rearrange("p (a h d) -> p a h d", a=NSQ, h=HPC),
                        COPY,
                    )
                return u

            def unit_KQ(qc, which):
                def u():
                    qsl = slice(qc * CH, (qc + 1) * CH)
                    wT = wkT if which == "k" else wqT
                    rots = rotk if which == "k" else rotq
                    psx = ps.tile(
                        [P, 2 * CH], F32, tag="work", bufs=2, name=f"ps{which}{qc}"
                    )
                    for dt in range(NDT):
                        for it in range(2):
                            nc.tensor.matmul(
                                psx[:, it * CH : (it + 1) * CH],
                                wT[:, dt, it * P : (it + 1) * P],
                                xt[dt][:, qsl],
                                start=(dt == 0),
                                stop=(dt == NDT - 1),
                            )
                    # DVE: psum->bf16 staging copy, then per-it shuffle,
                    # muls (bf16 2x), add. (StreamShuffle cannot convert
                    # dtype psum-f32 -> bf16: invalid ISA.)
                    qkb = rg.tile([P, 2 * CH], BF16, tag="qkb", bufs=2, name="qkb")
                    if which == "q":
                        # ACT stages Q's psum chunk: the DVE queue then runs
                        # K-chain -> Q-chain back-to-back with no copy between
                        nc.scalar.activation(qkb[:], psx[:], COPY)
                    else:
                        nc.vector.tensor_copy(qkb[:], psx[:])
                    for it in range(2):
                        qbi = qkb[:, it * CH : (it + 1) * CH]
                        sh = rg.tile([P, CH], BF16, tag="sh", bufs=4, name="sh")
                        nc.vector.stream_shuffle(sh[:], qbi, SHUF_MASK)
                        t1 = rg.tile([P, CH], BF16, tag="t1", bufs=3, name="t1")
                        nc.vector.tensor_mul(t1[:], qbi, cosT[:, qsl])
                        t2 = rg.tile([P, CH], BF16, tag="t2", bufs=3, name="t2")
                        nc.vector.tensor_mul(t2[:], sh[:], sinT[:, qsl])
                        nc.vector.tensor_add(rots[it][:, qsl], t1[:], t2[:])
                return u

            def b_qc_units(qc):
                """Attention for q-chunk qc: one unit per k-tile (scores+exp
                +mask, then PV of the PREVIOUS k-tile: 1-deep software
                pipeline so PV never heads the PE queue before its exp), then
                a finish unit (last PV + reciprocal + normalize -> attnT)."""
                nkt = (qc + 1) * KPC
                st_ = {"pend": []}

                def emit_pv(entries):
                    for (hp, kt, dj, pt2, ptm) in entries:
                        first = kt == st_["first_kt"]
                        last = kt == st_["last_kt"]
                        for half, hh in ((0, 2 * hp), (1, 2 * hp + 1)):
                            off = half * CH
                            lhs = vt[kt // KPC][:, kt % KPC, hh, :]
                            dst = st_["pso"][hp][:, off : off + CH]
                            if dj >= 0:
                                nc.tensor.matmul(
                                    dst[:, dj * P : (dj + 1) * P],
                                    lhs,
                                    ptm[:, half, :],
                                    start=first,
                                    stop=(last and dj == KPC - 1),
                                )
                                if dj < KPC - 1:
                                    nc.tensor.matmul(
                                        dst[:, (dj + 1) * P : CH],
                                        lhs,
                                        pt2[:, off + (dj + 1) * P : off + CH],
                                        start=False,
                                        stop=False,
                                    )
                            else:
                                nc.tensor.matmul(
                                    dst,
                                    lhs,
                                    pt2[:, off : off + CH],
                                    start=first,
                                    stop=last,
                                )

                def mk_kt(kt):
                    def u():
                        if kt == st_["first_kt"]:
                            st_["pso"] = [
                                ps.tile(
                                    [P, 2 * CH], F32, tag="pso", bufs=2,
                                    name=f"pso{hp}_{qc}",
                                )
                                for hp in range(2)
                            ]
                        dj = kt - KPC * qc
                        vs = max(0, dj) * P
                        ksl = slice(kt * P, (kt + 1) * P)
                        q0 = qc * CH + vs
                        q1 = (qc + 1) * CH
                        cur = []
                        for hp in range(2):
                            ss = ps.tile(
                                [P, 2 * CH], F32, tag="work", bufs=2,
                                name=f"ss{hp}_{qc}_{kt}",
                            )
                            nc.tensor.matmul(
                                ss[:, vs:CH],
                                rotk[hp][0:DK, ksl],
                                rotq[hp][0:DK, q0:q1],
                                start=True,
                                stop=True,
                            )
                            nc.tensor.matmul(
                                ss[:, CH + vs : 2 * CH],
                                rotk[hp][DK:P, ksl],
                                rotq[hp][DK:P, q0:q1],
                                start=True,
                                stop=True,
                            )
                            pt2 = rg.tile(
                                [P, 2 * CH], BF16, tag="pt2", bufs=14, name="pt2"
                            )
                            if vs == 0:
                                nc.scalar.activation(pt2[:], ss[:], EXP, scale=SCALE)
                            else:
                                s3 = ss[:].rearrange("p (h c) -> p h c", h=2)[
                                    :, :, vs:CH
                                ]
                                p3 = pt2[:].rearrange("p (h c) -> p h c", h=2)[
                                    :, :, vs:CH
                                ]
                                nc.scalar.activation(p3, s3, EXP, scale=SCALE)
                            ptm = None
                            if dj >= 0:
                                ptm = rg.tile(
                                    [P, 2, P], BF16, tag="ptm", bufs=14, name="ptm"
                                )
                                nc.vector.tensor_mul(
                                    ptm[:],
                                    pt2[:].rearrange("p (h c) -> p h c", h=2)[
                                        :, :, vs : vs + P
                                    ],
                                    dmT[:].unsqueeze(1).to_broadcast([P, 2, P]),
                                )
                            cur.append((hp, kt, dj, pt2, ptm))
                        st_["pend"].append(cur)
                        # 2-deep software pipeline: PV trails scores by 2
                        # k-tiles so exp latency never heads the PE queue.
                        # At the very last k-tile drain deeper so fin (recip/
                        # normalize -> o_proj tail) starts sooner.
                        depth = 1 if (qc == NQC - 1 and kt == st_["last_kt"]) else 5
                        while len(st_["pend"]) > depth:
                            emit_pv(st_["pend"].pop(0))
                    return u

                def fin():
                    at_ = rg.tile(
                        [P, 2, CH], BF16, tag="attnT", bufs=2, name=f"attnT{qc}"
                    )
                    attn_tiles[qc] = at_
                    pend = st_["pend"]
                    st_["pend"] = []
                    # per-hp: flush that hp's pending PV, then recip+normalize
                    # -> hp1's PV matmuls overlap hp0's reciprocal on DVE
                    for hp in range(2):
                        for ent in pend:
                            emit_pv([e for e in ent if e[0] == hp])
                        pso2 = st_["pso"][hp]
                        rden = rg.tile(
                            [DK, 2 * CH], F32, tag="rden", bufs=2, name="rden"
                        )
                        nc.vector.reciprocal_approx_fast(
                            out=rden[:], in_=pso2[0:DK, :]
                        )
                        nc.vector.tensor_mul(
                            at_[0:DK, hp, :], pso2[DK:P, 0:CH], rden[:, 0:CH]
                        )
                        nc.vector.tensor_mul(
                            at_[DK:P, hp, :],
                            pso2[DK:P, CH : 2 * CH],
                            rden[:, CH : 2 * CH],
                        )

                # process the DIAGONAL k-tiles first: the chunk's final PV
                # (which gates fin -> o_proj) is then a plain full-width
                # matmul with no tril-mask side chain on the tail
                diag0 = KPC * qc
                order = list(range(diag0, nkt)) + list(range(diag0))
                st_["first_kt"] = order[0]
                st_["last_kt"] = order[-1]
                return [mk_kt(kt) for kt in order], fin

            def mk_c(qc, st4, copy_eng="dve"):
                def u():
                    at_ = attn_tiles[qc]
                    psf = ps.tile(
                        [P, D], F32, tag="work", bufs=2, name=f"psf{qc}_{st4}"
                    )
                    # psum writes are capped at one bank (512 f32 cols):
                    # 2 column-halves x 2 it accumulation steps
                    for jc in range(2):
                        for it in range(2):
                            nc.tensor.matmul(
                                psf[:, jc * CH : (jc + 1) * CH],
                                at_[:, it, st4 * P : (st4 + 1) * P],
                                woT[:, it, jc * CH : (jc + 1) * CH],
                                start=(it == 0),
                                stop=(it == 1),
                            )
                    ost = rg.tile([P, D], BF16, tag="ost", bufs=4, name="ost")
                    if copy_eng == "act":
                        nc.scalar.activation(ost[:], psf[:], COPY)
                    else:
                        nc.vector.tensor_copy(ost[:], psf[:])
                    row0 = qc * CH + st4 * P
                    nc.sync.dma_start(out[row0 : row0 + P, :], ost[:])
                return u

            def weave(bu, afills, cfills, cfrac=0.45):
                """A-fills as a consecutive block ~15% in (their RoPE gates
                the next chunk); C-fills spread over units after cfrac."""
                n = len(bu)
                k = len(cfills)
                apos = max(1, int(n * 0.15))
                c0 = int(n * cfrac)
                span = max(1, n - c0)
                res = []
                fi = 0
                for i, b in enumerate(bu):
                    res.append(b)
                    if i + 1 == apos:
                        res += afills
                    if i >= c0:
                        want = (i - c0 + 1) * k // span
                        while fi < want:
                            res.append(cfills[fi])
                            fi += 1
                res += cfills[fi:]
                return res

            A = {qc: [unit_KQ(qc, "k"), unit_KQ(qc, "q"), unit_V(qc)]
                 for qc in range(NQC)}
            Cs = {qc: [mk_c(qc, s) for s in range(NSQ)] for qc in range(NQC - 1)}
            Cs[NQC - 1] = [
                mk_c(NQC - 1, s, copy_eng=("act" if s % 2 == 0 else "dve"))
                for s in range(NSQ)
            ]

            for u in A[0]:
                u()
            for qc in range(NQC):
                kts, fin = b_qc_units(qc)
                if qc == 0:
                    sched = weave(kts, A[1], [])
                elif qc < NQC - 1:
                    af = A[qc + 1]
                    cf = Cs[qc - 1] if qc == 1 else Cs[qc - 1][:2]
                    sched = weave(kts, af, cf, cfrac=0.65)
                else:
                    sched = weave(kts, [], Cs[qc - 2][2:] + Cs[qc - 1], cfrac=0.45)
                for u in sched:
                    u()
                fin()
            for u in Cs[NQC - 1]:
                u()

    nc.compile()
    return nc


def _tables():
    r = np.arange(P)
    j = 16 * ((r % 64) // 32) + (r % 16)
    inv = THETA ** (-(2.0 * j) / DK)
    ang = np.arange(S)[None, :] * inv[:, None]
    cosf = np.cos(ang).astype(np.float32)
    sgn = np.where((r % 32) < 16, -1.0, 1.0)
    sinf = (np.sin(ang) * sgn[:, None]).astype(np.float32)
    dmask = np.where(
        np.arange(P)[:, None] <= np.arange(P)[None, :], 1.0, 0.0
    )  # tril01: 1 where k <= q
    return cosf, sinf, dmask


def _head_perm():
    # sbuf row r (within a head) <- original head dim perm[r]:
    # windows of 32 rows = [16 even dims, 16 odd dims]
    r = np.arange(DK)
    w = r // 32
    idx = r % 32
    return np.where(idx < 16, 32 * w + 2 * idx, 32 * w + 2 * (idx - 16) + 1)


LAST_RESULTS = None


def kernel(**inputs):
    global LAST_RESULTS
    import ml_dtypes

    bf16 = ml_dtypes.bfloat16
    x = np.ascontiguousarray(np.asarray(inputs["in_features"], dtype=np.float32))
    qp = np.asarray(inputs["q_proj"], dtype=np.float32)
    kp = np.asarray(inputs["k_proj"], dtype=np.float32)
    vp = np.asarray(inputs["v_proj"], dtype=np.float32)
    op = np.asarray(inputs["o_proj"], dtype=np.float32)

    if "nc" not in _CACHE:
        _CACHE["nc"] = _build_nc()
        _CACHE["tables"] = _tables()
    nc = _CACHE["nc"]
    cosf, sinf, dmask = _CACHE["tables"]
    perm = _head_perm()
    idx = (np.arange(HPC)[:, None] * DK + perm[None, :]).reshape(-1)

    in_maps = []
    for c in range(NCORES):
        b, g = c // 4, c % 4
        rows = slice(HPC * g * DK, HPC * (g + 1) * DK)
        in_maps.append(
            {
                "x": np.ascontiguousarray(x[b].T.astype(bf16)),
                "wq": np.ascontiguousarray(qp[rows, :][idx, :].T.astype(bf16)),
                "wk": np.ascontiguousarray(kp[rows, :][idx, :].T.astype(bf16)),
                "wv": np.ascontiguousarray(vp[rows, :].T.astype(bf16)),
                "wo": np.ascontiguousarray(op[:, rows].T.astype(bf16)),
                "cosf": cosf.astype(bf16),
                "sinf": sinf.astype(bf16),
                "dmask": dmask.astype(bf16),
            }
        )

    res = run_bass_kernel_spmd(nc, in_maps, core_ids=list(range(NCORES)))
    LAST_RESULTS = res
    outp = np.zeros((B, S, D), dtype=np.float32)
    for c in range(NCORES):
        outp[c // 4] += res.results[c]["out"].astype(np.float32)
    return outp
